# revision 6
# baseline (speedup 1.0000x reference)
"""MoE (16 experts, top-2, shared SwiGLU expert) on 8 trn2 NeuronCores.

Strategy (expert-parallel, per sharding hint):
- Host computes the (cheap) gate: scores -> softmax -> top-2 -> combine
  weights. This determines the token->expert routing, i.e. the sharding.
- Each core c owns experts (2c, 2c+1). Tokens routed to an expert are
  gathered (host-side all-to-all) into a fixed-capacity [D, C] column
  matrix per expert, transposed so the contraction dim D sits on SBUF
  partitions.
- The shared expert is tensor-parallel over its hidden dim: core c owns
  rows [352c, 352(c+1)) of sw1/sw3 and the matching columns of sw2, and
  produces a [D, T] partial that the host sums over cores.
- Device math (per core): SwiGLU for 2 experts on their gathered tokens
  plus the shared-expert shard for all tokens. bf16 operands, fp32 PSUM
  accumulation. The per-token routing weight is applied host-side during
  the scatter-add (it commutes with the second matmul).
- Capacity overflow (count > C) is handled host-side for the few
  overflow tokens, so the kernel stays correct for any routing.

Outputs per core: yg [2, D, C] (routed, transposed), ysh [D, T] (shared
partial). Host: y = sum_c ysh_c^T + scatter-add of weighted yg columns.
"""

import numpy as np
import ml_dtypes

import concourse.bacc as bacc
import concourse.bass as bass
import concourse.mybir as mybir
import concourse.tile as tile
from concourse.bass_utils import run_bass_kernel_spmd

# problem shape (hardcoded per contract)
B, S, D = 2, 2048, 2048
T = B * S                  # 4096 tokens
E = 16                     # routed experts
TOPK = 2
H = 1408                   # expert hidden
HS = 2816                  # shared expert hidden
N_CORES = 8
E_LOC = E // N_CORES       # 2 experts per core
HSS = HS // N_CORES        # 352 shared-hidden rows per core

C = 576                    # per-expert token capacity (actual max ~567)
KD = D // 128              # 16 contraction tiles over D
KH = H // 128              # 11 contraction tiles over H
NT = T // 512              # 8 token blocks for the shared expert
T_TILES = [(0, 512), (512, 64)]          # routed token sub-tiles (C=576)
SH_HS = [(0, 128), (128, 128), (256, 96)]  # shared hidden sub-tiles (352)

BF = mybir.dt.bfloat16
F32 = mybir.dt.float32
BF_NP = ml_dtypes.bfloat16

_BUILD_CACHE = {}


def build_nc(timing_loop=True):
    """Build + compile the SPMD single-core program. Returns nc."""
    key = ("nc", timing_loop)
    if key in _BUILD_CACHE:
        return _BUILD_CACHE[key]

    nc = bacc.Bacc("TRN2", target_bir_lowering=False, debug=False)

    xT = nc.dram_tensor("xT", [D, T], BF, kind="ExternalInput").ap()
    xg = nc.dram_tensor("xg", [D, E_LOC * C], BF, kind="ExternalInput").ap()
    w1t = nc.dram_tensor("w1t", [E_LOC, KD, KH, 128, 128], BF, kind="ExternalInput").ap()
    w3t = nc.dram_tensor("w3t", [E_LOC, KD, KH, 128, 128], BF, kind="ExternalInput").ap()
    w2t = nc.dram_tensor("w2t", [E_LOC, KH, KD, 128, 128], BF, kind="ExternalInput").ap()
    sw1t = nc.dram_tensor("sw1t", [D, HSS], BF, kind="ExternalInput").ap()
    sw3t = nc.dram_tensor("sw3t", [D, HSS], BF, kind="ExternalInput").ap()
    sw2t = nc.dram_tensor("sw2t", [3, KD, 128, 128], BF, kind="ExternalInput").ap()
    niter = nc.dram_tensor("niter", [1, 1], mybir.dt.int32, kind="ExternalInput").ap()

    yg = nc.dram_tensor("yg", [E_LOC, D, C], F32, kind="ExternalOutput").ap()
    ysh = nc.dram_tensor("ysh", [D, T], F32, kind="ExternalOutput").ap()

    SIGMOID = mybir.ActivationFunctionType.Sigmoid

    import contextlib

    with tile.TileContext(nc) as tc:
        # runtime repeat count (timing support; harness runs with 1)
        if timing_loop:
            regs = []
            for _, eng in nc.engines.items():
                r = eng.alloc_register(f"niter_{_}")
                eng.reg_load(r, niter[0:1, 0:1])
                regs.append(r)
            rbound = bass.RegisterHandles(iter(regs))
            loop_cm = tc.For_i(0, rbound)
        else:
            loop_cm = contextlib.nullcontext()
        with loop_cm:
            with (
                tc.tile_pool(name="xgp", bufs=33) as xgp,
                tc.tile_pool(name="actp", bufs=13) as actp,
                tc.tile_pool(name="wsp", bufs=8) as wsp,
                tc.tile_pool(name="swp", bufs=16) as swp,
                tc.tile_pool(name="sw2p", bufs=48) as sw2p,
                tc.tile_pool(name="tmpp", bufs=4) as tmpp,
                tc.tile_pool(name="youtp", bufs=6) as youtp,
                tc.tile_pool(name="ps1", bufs=3, space="PSUM") as ps1,
                tc.tile_pool(name="ps2", bufs=2, space="PSUM") as ps2,
            ):
                # ---------------- routed experts ----------------
                for e in range(E_LOC):
                    xg_sb = []
                    for k in range(KD):
                        t = xgp.tile([128, C], BF, tag="xg", name="xg")
                        nc.sync.dma_start(
                            out=t, in_=xg[k * 128:(k + 1) * 128, e * C:(e + 1) * C]
                        )
                        xg_sb.append(t)

                    # mm1: h1 = w1^T-tile.T @ xg, h3 likewise; [H,C] layout
                    act_sb = []
                    for h in range(KH):
                        h1ps = [ps1.tile([128, 512], F32, tag="h1", name="h1") for _ in T_TILES]
                        h3ps = [ps1.tile([128, 512], F32, tag="h3", name="h3") for _ in T_TILES]
                        for k in range(KD):
                            w1s = wsp.tile([128, 128], BF, tag="w1s", name="w1s")
                            nc.sync.dma_start(out=w1s, in_=w1t[e, k, h])
                            for ti, (off, wd) in enumerate(T_TILES):
                                nc.tensor.matmul(
                                    h1ps[ti][:, :wd], w1s, xg_sb[k][:, off:off + wd],
                                    start=(k == 0), stop=(k == KD - 1),
                                )
                            w3s = wsp.tile([128, 128], BF, tag="w3s", name="w3s")
                            nc.sync.dma_start(out=w3s, in_=w3t[e, k, h])
                            for ti, (off, wd) in enumerate(T_TILES):
                                nc.tensor.matmul(
                                    h3ps[ti][:, :wd], w3s, xg_sb[k][:, off:off + wd],
                                    start=(k == 0), stop=(k == KD - 1),
                                )
                        act = actp.tile([128, C], BF, tag="act", name="act")
                        for ti, (off, wd) in enumerate(T_TILES):
                            tmp = tmpp.tile([128, 512], F32, tag="tmp", name="tmp")
                            nc.scalar.activation(tmp[:, :wd], h1ps[ti][:, :wd], SIGMOID)
                            nc.vector.tensor_mul(
                                tmp[:, :wd], tmp[:, :wd], h1ps[ti][:, :wd]
                            )
                            nc.vector.tensor_mul(
                                act[:, off:off + wd], tmp[:, :wd], h3ps[ti][:, :wd]
                            )
                        act_sb.append(act)

                    # mm2: y_e^T[d,t] = sum_h w2t-tile.T @ act
                    for d in range(KD):
                        yps = [ps2.tile([128, 512], F32, tag="yps", name="yps") for _ in T_TILES]
                        for h in range(KH):
                            w2s = wsp.tile([128, 128], BF, tag="w2s", name="w2s")
                            nc.sync.dma_start(out=w2s, in_=w2t[e, h, d])
                            for ti, (off, wd) in enumerate(T_TILES):
                                nc.tensor.matmul(
                                    yps[ti][:, :wd], w2s, act_sb[h][:, off:off + wd],
                                    start=(h == 0), stop=(h == KH - 1),
                                )
                        for ti, (off, wd) in enumerate(T_TILES):
                            yo = youtp.tile([128, 512], F32, tag="yout", name="yout")
                            nc.vector.tensor_copy(yo[:, :wd], yps[ti][:, :wd])
                            nc.sync.dma_start(
                                out=yg[e, d * 128:(d + 1) * 128, off:off + wd],
                                in_=yo[:, :wd],
                            )

                # ---------------- shared expert (TP shard) ----------------
                sw1_sb, sw3_sb = [], []
                for k in range(KD):
                    t1 = swp.tile([128, HSS], BF, tag="sw1", name="sw1")
                    nc.sync.dma_start(out=t1, in_=sw1t[k * 128:(k + 1) * 128, :])
                    sw1_sb.append(t1)
                    t3 = swp.tile([128, HSS], BF, tag="sw3", name="sw3")
                    nc.sync.dma_start(out=t3, in_=sw3t[k * 128:(k + 1) * 128, :])
                    sw3_sb.append(t3)
                sw2_sb = {}
                for hs in range(3):
                    for d in range(KD):
                        t = sw2p.tile([128, 128], BF, tag="sw2", name="sw2")
                        nc.sync.dma_start(out=t, in_=sw2t[hs, d])
                        sw2_sb[(hs, d)] = t

                for tb in range(NT):
                    toff = tb * 512
                    x_sb = []
                    for k in range(KD):
                        t = xgp.tile([128, 512], BF, tag="xg", name="xg")
                        nc.sync.dma_start(
                            out=t, in_=xT[k * 128:(k + 1) * 128, toff:toff + 512]
                        )
                        x_sb.append(t)

                    sh_sb = []
                    for hs, (hoff, m) in enumerate(SH_HS):
                        h1ps = ps1.tile([128, 512], F32, tag="h1", name="h1")
                        h3ps = ps1.tile([128, 512], F32, tag="h3", name="h3")
                        for k in range(KD):
                            nc.tensor.matmul(
                                h1ps[:m, :], sw1_sb[k][:, hoff:hoff + m], x_sb[k],
                                start=(k == 0), stop=(k == KD - 1),
                            )
                            nc.tensor.matmul(
                                h3ps[:m, :], sw3_sb[k][:, hoff:hoff + m], x_sb[k],
                                start=(k == 0), stop=(k == KD - 1),
                            )
                        sh = actp.tile([128, 512], BF, tag="act", name="act")
                        tmp = tmpp.tile([128, 512], F32, tag="tmp", name="tmp")
                        nc.scalar.activation(tmp[:m, :], h1ps[:m, :], SIGMOID)
                        nc.vector.tensor_mul(tmp[:m, :], tmp[:m, :], h1ps[:m, :])
                        nc.vector.tensor_mul(sh[:m, :], tmp[:m, :], h3ps[:m, :])
                        sh_sb.append(sh)

                    for d in range(KD):
                        yps = ps2.tile([128, 512], F32, tag="yps", name="yps")
                        for hs, (hoff, m) in enumerate(SH_HS):
                            nc.tensor.matmul(
                                yps, sw2_sb[(hs, d)][:m, :], sh_sb[hs][:m, :],
                                start=(hs == 0), stop=(hs == 2),
                            )
                        yo = youtp.tile([128, 512], F32, tag="yout", name="yout")
                        nc.vector.tensor_copy(yo, yps)
                        nc.sync.dma_start(
                            out=ysh[d * 128:(d + 1) * 128, toff:toff + 512], in_=yo
                        )

    nc.compile()
    _BUILD_CACHE[key] = nc
    return nc


def _routing(x2d, gate_w):
    """Host gate: returns (top2 idx [T,2], normalized top2 vals [T,2],
    combine weight matrix w [T,E], aux_loss)."""
    scores = x2d @ gate_w.T                       # fp32
    m = scores.max(-1, keepdims=True)
    p = np.exp(scores - m, dtype=np.float32)
    p = p / p.sum(-1, keepdims=True)
    top2 = np.argsort(-p, axis=-1, kind="stable")[:, :TOPK]
    tv = np.take_along_axis(p, top2, axis=-1)
    tvn = tv / (tv.sum(-1, keepdims=True) + 1e-9)
    w = np.zeros((x2d.shape[0], E), np.float32)
    np.put_along_axis(w, top2, tvn.astype(np.float32), axis=-1)
    importance = p.mean(0)
    load = w.sum(0) / (w.sum() + np.float32(1e-9))
    aux = np.float32(E) * np.float32((importance * load).sum())
    return top2, tvn, w, aux


def _tile_kxm(a):  # [R*128, Cc*128] -> [R/128? ...] block-tiled [r, c, 128, 128]
    r, c = a.shape
    return np.ascontiguousarray(
        a.reshape(r // 128, 128, c // 128, 128).swapaxes(1, 2)
    )


def prepare(x, gate_w, w1, w2, w3, sw1, sw2, sw3, niter=1):
    """Host shard prep. Returns (in_maps, meta) where meta has what's
    needed to combine results."""
    x2d = np.asarray(x, np.float32).reshape(T, D)
    top2, tvn, w, aux = _routing(x2d, np.asarray(gate_w, np.float32))

    xTb = np.ascontiguousarray(x2d.T).astype(BF_NP)

    # token lists per expert
    idx_e, overflow = [], []
    for e in range(E):
        idx = np.nonzero((top2 == e).any(-1))[0]
        if len(idx) > C:
            overflow.append((e, idx[C:]))
            idx = idx[:C]
        idx_e.append(idx)

    nit = np.array([[niter]], np.int32)
    in_maps = []
    for c in range(N_CORES):
        ge = [E_LOC * c + i for i in range(E_LOC)]
        gidx = np.zeros(E_LOC * C, np.int64)
        for i, g in enumerate(ge):
            gidx[i * C:i * C + len(idx_e[g])] = idx_e[g]
        in_maps.append({
            "xT": xTb,
            "xg": np.ascontiguousarray(xTb[:, gidx]),
            "w1t": np.stack([_tile_kxm(np.asarray(w1[g], np.float32).T.astype(BF_NP)) for g in ge]),
            "w3t": np.stack([_tile_kxm(np.asarray(w3[g], np.float32).T.astype(BF_NP)) for g in ge]),
            "w2t": np.stack([_tile_kxm(np.asarray(w2[g], np.float32).T.astype(BF_NP)) for g in ge]),
            "sw1t": np.ascontiguousarray(np.asarray(sw1, np.float32)[c * HSS:(c + 1) * HSS, :].T).astype(BF_NP),
            "sw3t": np.ascontiguousarray(np.asarray(sw3, np.float32)[c * HSS:(c + 1) * HSS, :].T).astype(BF_NP),
            "sw2t": _tile_kxm(np.vstack([
                np.ascontiguousarray(np.asarray(sw2, np.float32)[:, c * HSS:(c + 1) * HSS].T),
                np.zeros((3 * 128 - HSS, D), np.float32),
            ]).astype(BF_NP)),
            "niter": nit,
        })
    meta = dict(idx_e=idx_e, overflow=overflow, w=w, aux=aux, x2d=x2d)
    return in_maps, meta


def combine(results, meta, w1, w2, w3):
    idx_e, overflow, w = meta["idx_e"], meta["overflow"], meta["w"]
    y2d = np.zeros((T, D), np.float32)
    for c in range(N_CORES):
        y2d += results[c]["ysh"].T
        for i in range(E_LOC):
            g = E_LOC * c + i
            idx = idx_e[g]
            if len(idx) == 0:
                continue
            blk = results[c]["yg"][i][:, :len(idx)].T  # [cnt, D]
            y2d[idx] += blk * w[idx, g][:, None]
    # host fallback for capacity-overflow tokens (exact fp32)
    x2d = meta["x2d"]
    for g, idx in overflow:
        xs = x2d[idx]
        h1 = xs @ np.asarray(w1[g], np.float32).T
        h3 = xs @ np.asarray(w3[g], np.float32).T
        act = (h1 / (1.0 + np.exp(-h1))) * h3
        y2d[idx] += (act @ np.asarray(w2[g], np.float32).T) * w[idx, g][:, None]
    return y2d


def kernel(x, gate_w, w1, w2, w3, sw1, sw2, sw3):
    nc = build_nc()
    in_maps, meta = prepare(x, gate_w, w1, w2, w3, sw1, sw2, sw3, niter=1)
    res = run_bass_kernel_spmd(nc, in_maps, list(range(N_CORES)))
    y2d = combine(res.results, meta, w1, w2, w3)
    y = y2d.reshape(B, S, D)
    return y, meta["aux"]


# revision 14
# speedup vs baseline: 2.1322x; 2.1322x over previous
"""MoE (16 experts, top-2, shared SwiGLU expert) on 8 trn2 NeuronCores.

Strategy (expert-parallel, per sharding hint):
- Host computes the (cheap) gate: scores -> softmax -> top-2 -> combine
  weights. This determines the token->expert routing, i.e. the sharding.
- Each core c owns experts (2c, 2c+1). Tokens routed to an expert are
  gathered (host-side all-to-all) into a fixed-capacity [D, C] column
  matrix per expert, transposed so the contraction dim D sits on SBUF
  partitions.
- The shared expert is tensor-parallel over its hidden dim: core c owns
  rows [352c, 352(c+1)) of sw1/sw3 and the matching columns of sw2, and
  produces a [D, T] partial that the host sums over cores.
- Device math (per core): SwiGLU for 2 experts on their gathered tokens
  plus the shared-expert shard for all tokens. bf16 operands, fp32 PSUM
  accumulation. The per-token routing weight is applied host-side during
  the scatter-add (it commutes with the second matmul).
- Capacity overflow (count > C) is handled host-side for the few
  overflow tokens, so the kernel stays correct for any routing.

DMA-count discipline (HWDGE is a serialized ~625ns/DMA resource): every
input tensor is host-packed partition-major so it loads in one (or a
few) large multi-descriptor DMAs; output stores issue on the otherwise
idle GpSimd engine (SWDGE), keeping the HWDGE nearly free.

Outputs per core: yg [2, D, C] (routed, transposed), ysh [D, T] (shared
partial). Host: y = sum_c ysh_c^T + scatter-add of weighted yg columns.
"""

import contextlib

import numpy as np
import ml_dtypes

import concourse.bacc as bacc
import concourse.bass as bass
import concourse.mybir as mybir
import concourse.tile as tile
from concourse.bass_utils import run_bass_kernel_spmd

# problem shape (hardcoded per contract)
B, S, D = 2, 2048, 2048
T = B * S                  # 4096 tokens
E = 16                     # routed experts
TOPK = 2
H = 1408                   # expert hidden
HS = 2816                  # shared expert hidden
N_CORES = 8
E_LOC = E // N_CORES       # 2 experts per core
HSS = HS // N_CORES        # 352 shared-hidden rows per core

C = 576                    # per-expert token capacity (actual max ~567)
KD = D // 128              # 16 contraction tiles over D
KH = H // 128              # 11 contraction tiles over H
NT = T // 512              # 8 token blocks for the shared expert
T_TILES = [(0, 512), (512, 64)]            # routed token sub-tiles (C=576)
SH_HS = [(0, 128), (128, 128), (256, 96)]  # shared hidden sub-tiles (352)

BF = mybir.dt.bfloat16
F32 = mybir.dt.float32
BF_NP = ml_dtypes.bfloat16

_BUILD_CACHE = {}


def build_nc(timing_loop=True):
    """Build + compile the SPMD single-core program. Returns nc."""
    key = ("nc", timing_loop)
    if key in _BUILD_CACHE:
        return _BUILD_CACHE[key]

    nc = bacc.Bacc("TRN2", target_bir_lowering=False, debug=False)

    # host-packed, partition-major inputs (see prepare())
    w13 = nc.dram_tensor("w13", [E_LOC, 128, KD, 2, H], BF, kind="ExternalInput").ap()
    w2q = nc.dram_tensor("w2q", [E_LOC, KD, 128, KH, 128], BF, kind="ExternalInput").ap()
    xgq = nc.dram_tensor("xgq", [128, KD, E_LOC * C], BF, kind="ExternalInput").ap()
    xq = nc.dram_tensor("xq", [NT, 128, KD, 512], BF, kind="ExternalInput").ap()
    sw13 = nc.dram_tensor("sw13", [128, KD, 2, HSS], BF, kind="ExternalInput").ap()
    sw2q = nc.dram_tensor("sw2q", [128, 3, D], BF, kind="ExternalInput").ap()
    niter = nc.dram_tensor("niter", [1, 1], mybir.dt.int32, kind="ExternalInput").ap()

    yg = nc.dram_tensor("yg", [E_LOC, D, C], F32, kind="ExternalOutput").ap()
    ysh = nc.dram_tensor("ysh", [D, T], F32, kind="ExternalOutput").ap()

    SIGMOID = mybir.ActivationFunctionType.Sigmoid

    with tile.TileContext(nc) as tc:
        # runtime repeat count (timing support; harness runs with 1)
        if timing_loop:
            regs = []
            for ename, eng in nc.engines.items():
                r = eng.alloc_register(f"niter_{ename}")
                eng.reg_load(r, niter[0:1, 0:1])
                regs.append(r)
            rbound = bass.RegisterHandles(iter(regs))
            loop_cm = tc.For_i(0, rbound)
        else:
            loop_cm = contextlib.nullcontext()

        with loop_cm:
            with (
                tc.tile_pool(name="swp", bufs=1) as swp,
                tc.tile_pool(name="actp", bufs=13) as actp,
                tc.tile_pool(name="tmpp", bufs=3) as tmpp,
                tc.tile_pool(name="youtp", bufs=5) as youtp,
            ):
                # ---------------- routed experts ----------------
                with (
                    tc.tile_pool(name="w13p", bufs=1) as w13p,
                    tc.tile_pool(name="w2p", bufs=3) as w2p,
                    tc.tile_pool(name="xgp", bufs=1) as xgp,
                    tc.tile_pool(name="ps1", bufs=3, space="PSUM") as ps1,
                    tc.tile_pool(name="ps1b", bufs=2, space="PSUM") as ps1b,
                    tc.tile_pool(name="ps2", bufs=3, space="PSUM") as ps2,
                ):
                    HGRP = [(0, 2), (2, 3), (5, 3), (8, 3)]  # h-tile groups
                    xg_tiles = {}

                    def load_xg(e):
                        halves = []
                        for kh in range(2):
                            t = xgp.tile([128, KD // 2, C], BF,
                                         tag=f"xg{e}{kh}", name=f"xg{e}{kh}")
                            nc.sync.dma_start(
                                out=t,
                                in_=xgq[:, kh * 8:(kh + 1) * 8, e * C:(e + 1) * C],
                            )
                            halves.append(t)
                        xg_tiles[e] = halves

                    def load_w13_grp(e, gi):
                        g0, gn = HGRP[gi]
                        wtg = w13p.tile([128, KD, 2, gn * 128], BF,
                                        tag=f"w13{gi}", name=f"w13{gi}")
                        nc.sync.dma_start(
                            out=wtg,
                            in_=w13[e][:, :, :, g0 * 128:(g0 + gn) * 128],
                        )
                        return wtg

                    sw_sb_holder = []
                    # emit every weight/token load up front; pool slots and the
                    # scheduler pace the actual transfers
                    all_wts = []
                    for e in range(E_LOC):
                        wts = [load_w13_grp(e, 0)]
                        load_xg(e)
                        for gi in range(1, len(HGRP)):
                            wts.append(load_w13_grp(e, gi))
                        all_wts.append(wts)
                    t13 = swp.tile([128, KD, 2, HSS], BF, tag="sw13", name="sw13")
                    nc.sync.dma_start(out=t13, in_=sw13)
                    t2 = swp.tile([128, 3, D], BF, tag="sw2", name="sw2")
                    nc.sync.dma_start(out=t2, in_=sw2q)
                    sw_sb_holder[:] = [t13, t2]
                    for e in range(E_LOC):
                        wts = all_wts[e]

                        # mm1 + swiglu, [H, C] layout
                        act_sb = []
                        for h in range(KH):
                            h1ps = [ps1.tile([128, 512], F32, tag="h1", name="h1")
                                    for _ in T_TILES]
                            h3ps = [ps1b.tile([128, 512], F32, tag="h3", name="h3")
                                    for _ in T_TILES]
                            gi = next(i for i, (g0, gn) in enumerate(HGRP)
                                      if g0 <= h < g0 + gn)
                            wt = wts[gi]
                            hs_ = slice((h - HGRP[gi][0]) * 128,
                                        (h - HGRP[gi][0] + 1) * 128)
                            for k in range(KD):
                                xgt = xg_tiles[e][k // 8]
                                for ti, (off, wd) in enumerate(T_TILES):
                                    nc.tensor.matmul(
                                        h1ps[ti][:, :wd], wt[:, k, 0, hs_],
                                        xgt[:, k % 8, off:off + wd],
                                        start=(k == 0), stop=(k == KD - 1),
                                    )
                                for ti, (off, wd) in enumerate(T_TILES):
                                    nc.tensor.matmul(
                                        h3ps[ti][:, :wd], wt[:, k, 1, hs_],
                                        xgt[:, k % 8, off:off + wd],
                                        start=(k == 0), stop=(k == KD - 1),
                                    )
                            act = actp.tile([128, C], BF, tag="act", name="act")
                            for ti, (off, wd) in enumerate(T_TILES):
                                tmp = tmpp.tile([128, 512], F32, tag="tmp", name="tmp")
                                nc.scalar.activation(tmp[:, :wd], h1ps[ti][:, :wd], SIGMOID)
                                nc.vector.tensor_mul(
                                    tmp[:, :wd], tmp[:, :wd], h1ps[ti][:, :wd]
                                )
                                nc.vector.tensor_mul(
                                    act[:, off:off + wd], tmp[:, :wd], h3ps[ti][:, :wd]
                                )
                            act_sb.append(act)

                        # mm2: y_e^T[d,t] = sum_h w2-tile.T @ act
                        for d in range(KD):
                            w2s = w2p.tile([128, KH, 128], BF, tag="w2s", name="w2s")
                            nc.sync.dma_start(out=w2s, in_=w2q[e, d])
                            yps = [ps2.tile([128, 512], F32, tag="yps", name="yps")
                                   for _ in T_TILES]
                            for h in range(KH):
                                for ti, (off, wd) in enumerate(T_TILES):
                                    nc.tensor.matmul(
                                        yps[ti][:, :wd], w2s[:, h, :],
                                        act_sb[h][:, off:off + wd],
                                        start=(h == 0), stop=(h == KH - 1),
                                    )
                            for ti, (off, wd) in enumerate(T_TILES):
                                yo = youtp.tile([128, 512], F32, tag="yout", name="yout")
                                nc.vector.tensor_copy(yo[:, :wd], yps[ti][:, :wd])
                                nc.gpsimd.dma_start(
                                    out=yg[e, d * 128:(d + 1) * 128, off:off + wd],
                                    in_=yo[:, :wd],
                                )

                # ---------------- shared expert (TP shard) ----------------
                with (
                    tc.tile_pool(name="xshp", bufs=3) as xshp,
                    tc.tile_pool(name="ps1s", bufs=3, space="PSUM") as ps1s,
                    tc.tile_pool(name="ps2s", bufs=2, space="PSUM") as ps2s,
                ):
                    sw13_sb, sw2_sb = sw_sb_holder

                    def load_x(tb):
                        x_sb = xshp.tile([128, KD, 512], BF, tag="xsh", name="xsh")
                        nc.sync.dma_start(out=x_sb, in_=xq[tb])
                        return x_sb

                    def sh_mm1(x_sb):
                        sh_sb = []
                        for hs, (hoff, m) in enumerate(SH_HS):
                            h1ps = ps1s.tile([128, 512], F32, tag="h1s", name="h1s")
                            h3ps = ps1s.tile([128, 512], F32, tag="h3s", name="h3s")
                            hsl = slice(hoff, hoff + m)
                            for k in range(KD):
                                nc.tensor.matmul(
                                    h1ps[:m, :], sw13_sb[:, k, 0, hsl], x_sb[:, k, :],
                                    start=(k == 0), stop=(k == KD - 1),
                                )
                                nc.tensor.matmul(
                                    h3ps[:m, :], sw13_sb[:, k, 1, hsl], x_sb[:, k, :],
                                    start=(k == 0), stop=(k == KD - 1),
                                )
                            sh = actp.tile([128, 512], BF, tag="act", name="act")
                            tmp = tmpp.tile([128, 512], F32, tag="tmp", name="tmp")
                            nc.scalar.activation(tmp[:m, :], h1ps[:m, :], SIGMOID)
                            nc.vector.tensor_mul(tmp[:m, :], tmp[:m, :], h1ps[:m, :])
                            nc.vector.tensor_mul(sh[:m, :], tmp[:m, :], h3ps[:m, :])
                            sh_sb.append(sh)
                        return sh_sb

                    def sh_mm2(tb, sh_sb):
                        toff = tb * 512
                        for d in range(KD):
                            yps = ps2s.tile([128, 512], F32, tag="ypss", name="ypss")
                            dsl = slice(d * 128, (d + 1) * 128)
                            for hs, (hoff, m) in enumerate(SH_HS):
                                nc.tensor.matmul(
                                    yps, sw2_sb[:m, hs, dsl], sh_sb[hs][:m, :],
                                    start=(hs == 0), stop=(hs == 2),
                                )
                            yo = youtp.tile([128, 512], F32, tag="yout", name="yout")
                            if d % 2 == 0:
                                nc.scalar.copy(yo, yps)
                            else:
                                nc.vector.tensor_copy(yo, yps)
                            nc.gpsimd.dma_start(out=ysh[dsl, toff:toff + 512], in_=yo)

                    for tb in range(NT):
                        sh_mm2(tb, sh_mm1(load_x(tb)))

    nc.compile()
    _BUILD_CACHE[key] = nc
    return nc


def _routing(x2d, gate_w):
    """Host gate: top-2 of softmax(x @ gate_w.T), combine weights, aux."""
    scores = x2d @ gate_w.T                       # fp32
    m = scores.max(-1, keepdims=True)
    p = np.exp(scores - m, dtype=np.float32)
    p = p / p.sum(-1, keepdims=True)
    top2 = np.argsort(-p, axis=-1, kind="stable")[:, :TOPK]
    tv = np.take_along_axis(p, top2, axis=-1)
    tvn = tv / (tv.sum(-1, keepdims=True) + 1e-9)
    w = np.zeros((x2d.shape[0], E), np.float32)
    np.put_along_axis(w, top2, tvn.astype(np.float32), axis=-1)
    importance = p.mean(0)
    load = w.sum(0) / (w.sum() + np.float32(1e-9))
    aux = np.float32(E) * np.float32((importance * load).sum())
    return top2, tvn, w, aux


def _pmaj(a, kd):
    """[kd*128, F...] -> partition-major [128, kd, F...]"""
    return np.ascontiguousarray(a.reshape(kd, 128, *a.shape[1:]).swapaxes(0, 1))


def prepare(x, gate_w, w1, w2, w3, sw1, sw2, sw3, niter=1):
    """Host shard prep. Returns (in_maps, meta)."""
    x2d = np.asarray(x, np.float32).reshape(T, D)
    top2, tvn, w, aux = _routing(x2d, np.asarray(gate_w, np.float32))

    xTb = np.ascontiguousarray(x2d.T).astype(BF_NP)   # [D, T]

    idx_e, overflow = [], []
    for e in range(E):
        idx = np.nonzero((top2 == e).any(-1))[0]
        if len(idx) > C:
            overflow.append((e, idx[C:]))
            idx = idx[:C]
        idx_e.append(idx)

    # shared x, token-block-major: xq[tb, p, k, c] = x[tb*512+c, k*128+p]
    xq = np.ascontiguousarray(
        xTb.reshape(KD, 128, NT, 512).transpose(2, 1, 0, 3)
    )

    # sw13: [128, KD, 2, HSS]  (shared across cores except the shard slice)
    nit = np.array([[niter]], np.int32)
    in_maps = []
    for c in range(N_CORES):
        ge = [E_LOC * c + i for i in range(E_LOC)]
        gidx = np.zeros(E_LOC * C, np.int64)
        for i, g in enumerate(ge):
            gidx[i * C:i * C + len(idx_e[g])] = idx_e[g]
        xg = xTb[:, gidx]                               # [D, 2C]
        w13 = np.stack([
            np.stack([
                _pmaj(np.asarray(w1[g], np.float32).T.astype(BF_NP), KD),
                _pmaj(np.asarray(w3[g], np.float32).T.astype(BF_NP), KD),
            ], axis=2)
            for g in ge
        ])
        w2qa = np.stack([
            np.ascontiguousarray(
                np.asarray(w2[g], np.float32).T.astype(BF_NP)
                .reshape(KH, 128, KD, 128).transpose(2, 1, 0, 3)
            )
            for g in ge
        ])
        sw13 = np.stack([
            _pmaj(np.ascontiguousarray(
                np.asarray(m, np.float32)[c * HSS:(c + 1) * HSS, :].T
            ).astype(BF_NP), KD)
            for m in (sw1, sw3)
        ], axis=2)
        sw2t = np.vstack([
            np.ascontiguousarray(np.asarray(sw2, np.float32)[:, c * HSS:(c + 1) * HSS].T),
            np.zeros((3 * 128 - HSS, D), np.float32),
        ]).astype(BF_NP)
        sw2q = np.ascontiguousarray(sw2t.reshape(3, 128, D).swapaxes(0, 1))
        in_maps.append({
            "w13": w13,
            "w2q": w2qa,
            "xgq": _pmaj(xg, KD),
            "xq": xq,
            "sw13": sw13,
            "sw2q": sw2q,
            "niter": nit,
        })
    meta = dict(idx_e=idx_e, overflow=overflow, w=w, aux=aux, x2d=x2d)
    return in_maps, meta


def combine(results, meta, w1, w2, w3):
    idx_e, overflow, w = meta["idx_e"], meta["overflow"], meta["w"]
    y2d = np.zeros((T, D), np.float32)
    for c in range(N_CORES):
        y2d += results[c]["ysh"].T
        for i in range(E_LOC):
            g = E_LOC * c + i
            idx = idx_e[g]
            if len(idx) == 0:
                continue
            blk = results[c]["yg"][i][:, :len(idx)].T  # [cnt, D]
            y2d[idx] += blk * w[idx, g][:, None]
    # host fallback for capacity-overflow tokens (exact fp32)
    x2d = meta["x2d"]
    for g, idx in overflow:
        xs = x2d[idx]
        h1 = xs @ np.asarray(w1[g], np.float32).T
        h3 = xs @ np.asarray(w3[g], np.float32).T
        act = (h1 / (1.0 + np.exp(-h1))) * h3
        y2d[idx] += (act @ np.asarray(w2[g], np.float32).T) * w[idx, g][:, None]
    return y2d


def kernel(x, gate_w, w1, w2, w3, sw1, sw2, sw3):
    nc = build_nc()
    in_maps, meta = prepare(x, gate_w, w1, w2, w3, sw1, sw2, sw3, niter=1)
    res = run_bass_kernel_spmd(nc, in_maps, list(range(N_CORES)))
    y2d = combine(res.results, meta, w1, w2, w3)
    y = y2d.reshape(B, S, D)
    return y, meta["aux"]


# revision 16
# speedup vs baseline: 2.2311x; 1.0464x over previous
"""MoE (16 experts, top-2, shared SwiGLU expert) on 8 trn2 NeuronCores.

Strategy (expert-parallel, per sharding hint):
- Host computes the (cheap) gate: scores -> softmax -> top-2 -> combine
  weights. This determines the token->expert routing, i.e. the sharding.
- Each core c owns experts (2c, 2c+1). Tokens routed to an expert are
  gathered (host-side all-to-all) into a fixed-capacity [D, C] column
  matrix per expert, transposed so the contraction dim D sits on SBUF
  partitions.
- The shared expert is tensor-parallel over its hidden dim: core c owns
  rows [352c, 352(c+1)) of sw1/sw3 and the matching columns of sw2, and
  produces a [D, T] partial that the host sums over cores.
- Device math (per core): SwiGLU for 2 experts on their gathered tokens
  plus the shared-expert shard for all tokens. bf16 operands, fp32 PSUM
  accumulation. The per-token routing weight is applied host-side during
  the scatter-add (it commutes with the second matmul).
- Capacity overflow (count > C) is handled host-side for the few
  overflow tokens, so the kernel stays correct for any routing.

DMA-count discipline (HWDGE is a serialized ~625ns/DMA resource): every
input tensor is host-packed partition-major so it loads in one (or a
few) large multi-descriptor DMAs; output stores issue on the otherwise
idle GpSimd engine (SWDGE), keeping the HWDGE nearly free.

Outputs per core: yg [2, D, C] (routed, transposed), ysh [D, T] (shared
partial). Host: y = sum_c ysh_c^T + scatter-add of weighted yg columns.
"""

import contextlib

import numpy as np
import ml_dtypes

import concourse.bacc as bacc
import concourse.bass as bass
import concourse.mybir as mybir
import concourse.tile as tile
from concourse.bass_utils import run_bass_kernel_spmd

# problem shape (hardcoded per contract)
B, S, D = 2, 2048, 2048
T = B * S                  # 4096 tokens
E = 16                     # routed experts
TOPK = 2
H = 1408                   # expert hidden
HS = 2816                  # shared expert hidden
N_CORES = 8
E_LOC = E // N_CORES       # 2 experts per core
HSS = HS // N_CORES        # 352 shared-hidden rows per core

C = 512                    # per-expert token capacity (= one 512-wide tile;
                           # overflow tokens are handled exactly on the host)
KD = D // 128              # 16 contraction tiles over D
KH = H // 128              # 11 contraction tiles over H
NT = T // 512              # 8 token blocks for the shared expert
T_TILES = [(0, 512)]                       # routed token sub-tiles (C=512)
SH_HS = [(0, 128), (128, 128), (256, 96)]  # shared hidden sub-tiles (352)

BF = mybir.dt.bfloat16
F32 = mybir.dt.float32
BF_NP = ml_dtypes.bfloat16

_BUILD_CACHE = {}


def build_nc(timing_loop=True):
    """Build + compile the SPMD single-core program. Returns nc."""
    key = ("nc", timing_loop)
    if key in _BUILD_CACHE:
        return _BUILD_CACHE[key]

    nc = bacc.Bacc("TRN2", target_bir_lowering=False, debug=False)

    # host-packed, partition-major inputs (see prepare())
    w13 = nc.dram_tensor("w13", [E_LOC, 128, KD, 2, H], BF, kind="ExternalInput").ap()
    w2q = nc.dram_tensor("w2q", [E_LOC, KD, 128, KH, 128], BF, kind="ExternalInput").ap()
    xgq = nc.dram_tensor("xgq", [128, KD, E_LOC * C], BF, kind="ExternalInput").ap()
    xq = nc.dram_tensor("xq", [NT, 128, KD, 512], BF, kind="ExternalInput").ap()
    sw13 = nc.dram_tensor("sw13", [128, KD, 2, HSS], BF, kind="ExternalInput").ap()
    sw2q = nc.dram_tensor("sw2q", [128, 3, D], BF, kind="ExternalInput").ap()
    niter = nc.dram_tensor("niter", [1, 1], mybir.dt.int32, kind="ExternalInput").ap()

    yg = nc.dram_tensor("yg", [E_LOC, D, C], F32, kind="ExternalOutput").ap()
    ysh = nc.dram_tensor("ysh", [D, T], F32, kind="ExternalOutput").ap()

    SIGMOID = mybir.ActivationFunctionType.Sigmoid

    with tile.TileContext(nc) as tc:
        # runtime repeat count (timing support; harness runs with 1)
        if timing_loop:
            regs = []
            for ename, eng in nc.engines.items():
                r = eng.alloc_register(f"niter_{ename}")
                eng.reg_load(r, niter[0:1, 0:1])
                regs.append(r)
            rbound = bass.RegisterHandles(iter(regs))
            loop_cm = tc.For_i(0, rbound)
        else:
            loop_cm = contextlib.nullcontext()

        with loop_cm:
            with (
                tc.tile_pool(name="swp", bufs=1) as swp,
                tc.tile_pool(name="actp", bufs=13) as actp,
                tc.tile_pool(name="tmpp", bufs=3) as tmpp,
                tc.tile_pool(name="youtp", bufs=5) as youtp,
            ):
                # ---------------- routed experts ----------------
                with (
                    tc.tile_pool(name="w13p", bufs=1) as w13p,
                    tc.tile_pool(name="w2p", bufs=3) as w2p,
                    tc.tile_pool(name="xgp", bufs=1) as xgp,
                    tc.tile_pool(name="ps1", bufs=3, space="PSUM") as ps1,
                    tc.tile_pool(name="ps1b", bufs=3, space="PSUM") as ps1b,
                    tc.tile_pool(name="ps2", bufs=2, space="PSUM") as ps2,
                ):
                    HGRP = [(0, 2), (2, 3), (5, 3), (8, 3)]  # h-tile groups
                    xg_tiles = {}

                    def load_xg(e):
                        halves = []
                        for kh in range(2):
                            t = xgp.tile([128, KD // 2, C], BF,
                                         tag=f"xg{e}{kh}", name=f"xg{e}{kh}")
                            nc.sync.dma_start(
                                out=t,
                                in_=xgq[:, kh * 8:(kh + 1) * 8, e * C:(e + 1) * C],
                            )
                            halves.append(t)
                        xg_tiles[e] = halves

                    def load_w13_grp(e, gi):
                        g0, gn = HGRP[gi]
                        wtg = w13p.tile([128, KD, 2, gn * 128], BF,
                                        tag=f"w13{gi}", name=f"w13{gi}")
                        nc.sync.dma_start(
                            out=wtg,
                            in_=w13[e][:, :, :, g0 * 128:(g0 + gn) * 128],
                        )
                        return wtg

                    sw_sb_holder = []
                    # emit every weight/token load up front; pool slots and the
                    # scheduler pace the actual transfers
                    all_wts = []
                    for e in range(E_LOC):
                        wts = [load_w13_grp(e, 0)]
                        load_xg(e)
                        for gi in range(1, len(HGRP)):
                            wts.append(load_w13_grp(e, gi))
                        all_wts.append(wts)
                    t13 = swp.tile([128, KD, 2, HSS], BF, tag="sw13", name="sw13")
                    nc.sync.dma_start(out=t13, in_=sw13)
                    t2 = swp.tile([128, 3, D], BF, tag="sw2", name="sw2")
                    nc.sync.dma_start(out=t2, in_=sw2q)
                    sw_sb_holder[:] = [t13, t2]
                    for e in range(E_LOC):
                        wts = all_wts[e]

                        # mm1 + swiglu, [H, C] layout
                        act_sb = []
                        for h in range(KH):
                            h1ps = [ps1.tile([128, 512], F32, tag="h1", name="h1")
                                    for _ in T_TILES]
                            h3ps = [ps1b.tile([128, 512], F32, tag="h3", name="h3")
                                    for _ in T_TILES]
                            gi = next(i for i, (g0, gn) in enumerate(HGRP)
                                      if g0 <= h < g0 + gn)
                            wt = wts[gi]
                            hs_ = slice((h - HGRP[gi][0]) * 128,
                                        (h - HGRP[gi][0] + 1) * 128)
                            for k in range(KD):
                                xgt = xg_tiles[e][k // 8]
                                for ti, (off, wd) in enumerate(T_TILES):
                                    nc.tensor.matmul(
                                        h1ps[ti][:, :wd], wt[:, k, 0, hs_],
                                        xgt[:, k % 8, off:off + wd],
                                        start=(k == 0), stop=(k == KD - 1),
                                    )
                                for ti, (off, wd) in enumerate(T_TILES):
                                    nc.tensor.matmul(
                                        h3ps[ti][:, :wd], wt[:, k, 1, hs_],
                                        xgt[:, k % 8, off:off + wd],
                                        start=(k == 0), stop=(k == KD - 1),
                                    )
                            act = actp.tile([128, C], BF, tag="act", name="act")
                            for ti, (off, wd) in enumerate(T_TILES):
                                tmp = tmpp.tile([128, 512], F32, tag="tmp", name="tmp")
                                nc.scalar.activation(tmp[:, :wd], h1ps[ti][:, :wd], SIGMOID)
                                nc.vector.tensor_mul(
                                    tmp[:, :wd], tmp[:, :wd], h1ps[ti][:, :wd]
                                )
                                nc.vector.tensor_mul(
                                    act[:, off:off + wd], tmp[:, :wd], h3ps[ti][:, :wd]
                                )
                            act_sb.append(act)

                        # mm2: y_e^T[d,t] = sum_h w2-tile.T @ act
                        for d in range(KD):
                            w2s = w2p.tile([128, KH, 128], BF, tag="w2s", name="w2s")
                            nc.sync.dma_start(out=w2s, in_=w2q[e, d])
                            yps = [ps2.tile([128, 512], F32, tag="yps", name="yps")
                                   for _ in T_TILES]
                            for h in range(KH):
                                for ti, (off, wd) in enumerate(T_TILES):
                                    nc.tensor.matmul(
                                        yps[ti][:, :wd], w2s[:, h, :],
                                        act_sb[h][:, off:off + wd],
                                        start=(h == 0), stop=(h == KH - 1),
                                    )
                            for ti, (off, wd) in enumerate(T_TILES):
                                yo = youtp.tile([128, 512], F32, tag="yout", name="yout")
                                nc.vector.tensor_copy(yo[:, :wd], yps[ti][:, :wd])
                                nc.gpsimd.dma_start(
                                    out=yg[e, d * 128:(d + 1) * 128, off:off + wd],
                                    in_=yo[:, :wd],
                                )

                # ---------------- shared expert (TP shard) ----------------
                with (
                    tc.tile_pool(name="xshp", bufs=3) as xshp,
                    tc.tile_pool(name="ps1s", bufs=3, space="PSUM") as ps1s,
                    tc.tile_pool(name="ps2s", bufs=2, space="PSUM") as ps2s,
                ):
                    sw13_sb, sw2_sb = sw_sb_holder

                    def load_x(tb):
                        x_sb = xshp.tile([128, KD, 512], BF, tag="xsh", name="xsh")
                        nc.sync.dma_start(out=x_sb, in_=xq[tb])
                        return x_sb

                    def sh_mm1(x_sb):
                        sh_sb = []
                        for hs, (hoff, m) in enumerate(SH_HS):
                            h1ps = ps1s.tile([128, 512], F32, tag="h1s", name="h1s")
                            h3ps = ps1s.tile([128, 512], F32, tag="h3s", name="h3s")
                            hsl = slice(hoff, hoff + m)
                            for k in range(KD):
                                nc.tensor.matmul(
                                    h1ps[:m, :], sw13_sb[:, k, 0, hsl], x_sb[:, k, :],
                                    start=(k == 0), stop=(k == KD - 1),
                                )
                                nc.tensor.matmul(
                                    h3ps[:m, :], sw13_sb[:, k, 1, hsl], x_sb[:, k, :],
                                    start=(k == 0), stop=(k == KD - 1),
                                )
                            sh = actp.tile([128, 512], BF, tag="act", name="act")
                            tmp = tmpp.tile([128, 512], F32, tag="tmp", name="tmp")
                            nc.scalar.activation(tmp[:m, :], h1ps[:m, :], SIGMOID)
                            nc.vector.tensor_mul(tmp[:m, :], tmp[:m, :], h1ps[:m, :])
                            nc.vector.tensor_mul(sh[:m, :], tmp[:m, :], h3ps[:m, :])
                            sh_sb.append(sh)
                        return sh_sb

                    def sh_mm2(tb, sh_sb):
                        toff = tb * 512
                        for d in range(KD):
                            yps = ps2s.tile([128, 512], F32, tag="ypss", name="ypss")
                            dsl = slice(d * 128, (d + 1) * 128)
                            for hs, (hoff, m) in enumerate(SH_HS):
                                nc.tensor.matmul(
                                    yps, sw2_sb[:m, hs, dsl], sh_sb[hs][:m, :],
                                    start=(hs == 0), stop=(hs == 2),
                                )
                            yo = youtp.tile([128, 512], F32, tag="yout", name="yout")
                            if d % 2 == 0:
                                nc.scalar.copy(yo, yps)
                            else:
                                nc.vector.tensor_copy(yo, yps)
                            nc.gpsimd.dma_start(out=ysh[dsl, toff:toff + 512], in_=yo)

                    for tb in range(NT):
                        sh_mm2(tb, sh_mm1(load_x(tb)))

    nc.compile()
    _BUILD_CACHE[key] = nc
    return nc


def _routing(x2d, gate_w):
    """Host gate: top-2 of softmax(x @ gate_w.T), combine weights, aux."""
    scores = x2d @ gate_w.T                       # fp32
    m = scores.max(-1, keepdims=True)
    p = np.exp(scores - m, dtype=np.float32)
    p = p / p.sum(-1, keepdims=True)
    top2 = np.argsort(-p, axis=-1, kind="stable")[:, :TOPK]
    tv = np.take_along_axis(p, top2, axis=-1)
    tvn = tv / (tv.sum(-1, keepdims=True) + 1e-9)
    w = np.zeros((x2d.shape[0], E), np.float32)
    np.put_along_axis(w, top2, tvn.astype(np.float32), axis=-1)
    importance = p.mean(0)
    load = w.sum(0) / (w.sum() + np.float32(1e-9))
    aux = np.float32(E) * np.float32((importance * load).sum())
    return top2, tvn, w, aux


def _pmaj(a, kd):
    """[kd*128, F...] -> partition-major [128, kd, F...]"""
    return np.ascontiguousarray(a.reshape(kd, 128, *a.shape[1:]).swapaxes(0, 1))


def prepare(x, gate_w, w1, w2, w3, sw1, sw2, sw3, niter=1):
    """Host shard prep. Returns (in_maps, meta)."""
    x2d = np.asarray(x, np.float32).reshape(T, D)
    top2, tvn, w, aux = _routing(x2d, np.asarray(gate_w, np.float32))

    xTb = np.ascontiguousarray(x2d.T).astype(BF_NP)   # [D, T]

    idx_e, overflow = [], []
    for e in range(E):
        idx = np.nonzero((top2 == e).any(-1))[0]
        if len(idx) > C:
            overflow.append((e, idx[C:]))
            idx = idx[:C]
        idx_e.append(idx)

    # shared x, token-block-major: xq[tb, p, k, c] = x[tb*512+c, k*128+p]
    xq = np.ascontiguousarray(
        xTb.reshape(KD, 128, NT, 512).transpose(2, 1, 0, 3)
    )

    # sw13: [128, KD, 2, HSS]  (shared across cores except the shard slice)
    nit = np.array([[niter]], np.int32)
    in_maps = []
    for c in range(N_CORES):
        ge = [E_LOC * c + i for i in range(E_LOC)]
        gidx = np.zeros(E_LOC * C, np.int64)
        for i, g in enumerate(ge):
            gidx[i * C:i * C + len(idx_e[g])] = idx_e[g]
        xg = xTb[:, gidx]                               # [D, 2C]
        w13 = np.stack([
            np.stack([
                _pmaj(np.asarray(w1[g], np.float32).T.astype(BF_NP), KD),
                _pmaj(np.asarray(w3[g], np.float32).T.astype(BF_NP), KD),
            ], axis=2)
            for g in ge
        ])
        w2qa = np.stack([
            np.ascontiguousarray(
                np.asarray(w2[g], np.float32).T.astype(BF_NP)
                .reshape(KH, 128, KD, 128).transpose(2, 1, 0, 3)
            )
            for g in ge
        ])
        sw13 = np.stack([
            _pmaj(np.ascontiguousarray(
                np.asarray(m, np.float32)[c * HSS:(c + 1) * HSS, :].T
            ).astype(BF_NP), KD)
            for m in (sw1, sw3)
        ], axis=2)
        sw2t = np.vstack([
            np.ascontiguousarray(np.asarray(sw2, np.float32)[:, c * HSS:(c + 1) * HSS].T),
            np.zeros((3 * 128 - HSS, D), np.float32),
        ]).astype(BF_NP)
        sw2q = np.ascontiguousarray(sw2t.reshape(3, 128, D).swapaxes(0, 1))
        in_maps.append({
            "w13": w13,
            "w2q": w2qa,
            "xgq": _pmaj(xg, KD),
            "xq": xq,
            "sw13": sw13,
            "sw2q": sw2q,
            "niter": nit,
        })
    meta = dict(idx_e=idx_e, overflow=overflow, w=w, aux=aux, x2d=x2d)
    return in_maps, meta


def combine(results, meta, w1, w2, w3):
    idx_e, overflow, w = meta["idx_e"], meta["overflow"], meta["w"]
    y2d = np.zeros((T, D), np.float32)
    for c in range(N_CORES):
        y2d += results[c]["ysh"].T
        for i in range(E_LOC):
            g = E_LOC * c + i
            idx = idx_e[g]
            if len(idx) == 0:
                continue
            blk = results[c]["yg"][i][:, :len(idx)].T  # [cnt, D]
            y2d[idx] += blk * w[idx, g][:, None]
    # host fallback for capacity-overflow tokens (exact fp32)
    x2d = meta["x2d"]
    for g, idx in overflow:
        xs = x2d[idx]
        h1 = xs @ np.asarray(w1[g], np.float32).T
        h3 = xs @ np.asarray(w3[g], np.float32).T
        act = (h1 / (1.0 + np.exp(-h1))) * h3
        y2d[idx] += (act @ np.asarray(w2[g], np.float32).T) * w[idx, g][:, None]
    return y2d


def kernel(x, gate_w, w1, w2, w3, sw1, sw2, sw3):
    nc = build_nc()
    in_maps, meta = prepare(x, gate_w, w1, w2, w3, sw1, sw2, sw3, niter=1)
    res = run_bass_kernel_spmd(nc, in_maps, list(range(N_CORES)))
    y2d = combine(res.results, meta, w1, w2, w3)
    y = y2d.reshape(B, S, D)
    return y, meta["aux"]


# revision 17
# speedup vs baseline: 2.4202x; 1.0848x over previous
"""MoE (16 experts, top-2, shared SwiGLU expert) on 8 trn2 NeuronCores.

Strategy (expert-parallel, per sharding hint):
- Host computes the (cheap) gate: scores -> softmax -> top-2 -> combine
  weights. This determines the token->expert routing, i.e. the sharding.
- Each core c owns experts (2c, 2c+1). Tokens routed to an expert are
  gathered (host-side all-to-all) into a fixed-capacity [D, C] column
  matrix per expert, transposed so the contraction dim D sits on SBUF
  partitions.
- The shared expert is tensor-parallel over its hidden dim: core c owns
  rows [352c, 352(c+1)) of sw1/sw3 and the matching columns of sw2, and
  produces a [D, T] partial that the host sums over cores.
- Device math (per core): SwiGLU for 2 experts on their gathered tokens
  plus the shared-expert shard for all tokens. bf16 operands, fp32 PSUM
  accumulation. The per-token routing weight is applied host-side during
  the scatter-add (it commutes with the second matmul).
- Capacity overflow (count > C) is handled host-side for the few
  overflow tokens, so the kernel stays correct for any routing.

DMA-count discipline (HWDGE is a serialized ~625ns/DMA resource): every
input tensor is host-packed partition-major so it loads in one (or a
few) large multi-descriptor DMAs; output stores issue on the otherwise
idle GpSimd engine (SWDGE), keeping the HWDGE nearly free.

Outputs per core: yg [2, D, C] (routed, transposed), ysh [D, T] (shared
partial). Host: y = sum_c ysh_c^T + scatter-add of weighted yg columns.
"""

import contextlib

import numpy as np
import ml_dtypes

import concourse.bacc as bacc
import concourse.bass as bass
import concourse.mybir as mybir
import concourse.tile as tile
from concourse.bass_utils import run_bass_kernel_spmd

# problem shape (hardcoded per contract)
B, S, D = 2, 2048, 2048
T = B * S                  # 4096 tokens
E = 16                     # routed experts
TOPK = 2
H = 1408                   # expert hidden
HS = 2816                  # shared expert hidden
N_CORES = 8
E_LOC = E // N_CORES       # 2 experts per core
HSS = HS // N_CORES        # 352 shared-hidden rows per core

C = 512                    # per-expert token capacity (= one 512-wide tile;
                           # overflow tokens are handled exactly on the host)
KD = D // 128              # 16 contraction tiles over D
KH = H // 128              # 11 contraction tiles over H
NT = T // 512              # 8 token blocks for the shared expert
T_TILES = [(0, 512)]                       # routed token sub-tiles (C=512)
SH_HS = [(0, 128), (128, 128), (256, 96)]  # shared hidden sub-tiles (352)

BF = mybir.dt.bfloat16
F32 = mybir.dt.float32
BF_NP = ml_dtypes.bfloat16

_BUILD_CACHE = {}


def build_nc(timing_loop=True):
    """Build + compile the SPMD single-core program. Returns nc."""
    key = ("nc", timing_loop)
    if key in _BUILD_CACHE:
        return _BUILD_CACHE[key]

    nc = bacc.Bacc("TRN2", target_bir_lowering=False, debug=False)

    # host-packed, partition-major inputs (see prepare())
    w13 = nc.dram_tensor("w13", [E_LOC, 128, KD, 2, H], BF, kind="ExternalInput").ap()
    w2q = nc.dram_tensor("w2q", [E_LOC, KD, 128, KH, 128], BF, kind="ExternalInput").ap()
    xgq = nc.dram_tensor("xgq", [128, KD, E_LOC * C], BF, kind="ExternalInput").ap()
    xq = nc.dram_tensor("xq", [NT, 128, KD, 512], BF, kind="ExternalInput").ap()
    sw13 = nc.dram_tensor("sw13", [128, KD, 2, HSS], BF, kind="ExternalInput").ap()
    sw2q = nc.dram_tensor("sw2q", [128, 3, D], BF, kind="ExternalInput").ap()
    niter = nc.dram_tensor("niter", [1, 1], mybir.dt.int32, kind="ExternalInput").ap()

    yg = nc.dram_tensor("yg", [E_LOC, D, C], F32, kind="ExternalOutput").ap()
    ysh = nc.dram_tensor("ysh", [D, T], F32, kind="ExternalOutput").ap()

    SIGMOID = mybir.ActivationFunctionType.Sigmoid

    with tile.TileContext(nc) as tc:
        # runtime repeat count (timing support; harness runs with 1)
        if timing_loop:
            regs = []
            for ename, eng in nc.engines.items():
                r = eng.alloc_register(f"niter_{ename}")
                eng.reg_load(r, niter[0:1, 0:1])
                regs.append(r)
            rbound = bass.RegisterHandles(iter(regs))
            loop_cm = tc.For_i(0, rbound)
        else:
            loop_cm = contextlib.nullcontext()

        with loop_cm:
            with (
                tc.tile_pool(name="swp", bufs=1) as swp,
                tc.tile_pool(name="actp", bufs=13) as actp,
                tc.tile_pool(name="tmpp", bufs=3) as tmpp,
                tc.tile_pool(name="youtp", bufs=5) as youtp,
            ):
                # ---------------- routed experts ----------------
                with (
                    tc.tile_pool(name="w13p", bufs=1) as w13p,
                    tc.tile_pool(name="w2p", bufs=3) as w2p,
                    tc.tile_pool(name="xgp", bufs=1) as xgp,
                    tc.tile_pool(name="ps1", bufs=3, space="PSUM") as ps1,
                    tc.tile_pool(name="ps1b", bufs=3, space="PSUM") as ps1b,
                    tc.tile_pool(name="ps2", bufs=2, space="PSUM") as ps2,
                ):
                    HGRP = [(0, 1), (1, 3), (4, 4), (8, 3)]  # h-tile groups
                    xg_tiles = {}

                    def load_xg(e):
                        parts = []
                        for kh in range(4):
                            t = xgp.tile([128, KD // 4, C], BF,
                                         tag=f"xg{e}{kh}", name=f"xg{e}{kh}")
                            nc.sync.dma_start(
                                out=t,
                                in_=xgq[:, kh * 4:(kh + 1) * 4, e * C:(e + 1) * C],
                            )
                            parts.append(t)
                        xg_tiles[e] = parts

                    def load_w13_grp(e, gi):
                        g0, gn = HGRP[gi]
                        wtg = w13p.tile([128, KD, 2, gn * 128], BF,
                                        tag=f"w13{gi}", name=f"w13{gi}")
                        nc.sync.dma_start(
                            out=wtg,
                            in_=w13[e][:, :, :, g0 * 128:(g0 + gn) * 128],
                        )
                        return wtg

                    sw_sb_holder = []
                    # emit every weight/token load up front; pool slots and the
                    # scheduler pace the actual transfers
                    all_wts = []
                    for e in range(E_LOC):
                        wts = [load_w13_grp(e, 0)]
                        load_xg(e)
                        for gi in range(1, len(HGRP)):
                            wts.append(load_w13_grp(e, gi))
                        all_wts.append(wts)
                    t13 = swp.tile([128, KD, 2, HSS], BF, tag="sw13", name="sw13")
                    nc.sync.dma_start(out=t13, in_=sw13)
                    t2 = swp.tile([128, 3, D], BF, tag="sw2", name="sw2")
                    nc.sync.dma_start(out=t2, in_=sw2q)
                    sw_sb_holder[:] = [t13, t2]
                    for e in range(E_LOC):
                        wts = all_wts[e]

                        # mm1 + swiglu, [H, C] layout
                        act_sb = []
                        for h in range(KH):
                            h1ps = [ps1.tile([128, 512], F32, tag="h1", name="h1")
                                    for _ in T_TILES]
                            h3ps = [ps1b.tile([128, 512], F32, tag="h3", name="h3")
                                    for _ in T_TILES]
                            gi = next(i for i, (g0, gn) in enumerate(HGRP)
                                      if g0 <= h < g0 + gn)
                            wt = wts[gi]
                            hs_ = slice((h - HGRP[gi][0]) * 128,
                                        (h - HGRP[gi][0] + 1) * 128)
                            for k in range(KD):
                                xgt = xg_tiles[e][k // 4]
                                for ti, (off, wd) in enumerate(T_TILES):
                                    nc.tensor.matmul(
                                        h1ps[ti][:, :wd], wt[:, k, 0, hs_],
                                        xgt[:, k % 4, off:off + wd],
                                        start=(k == 0), stop=(k == KD - 1),
                                    )
                                for ti, (off, wd) in enumerate(T_TILES):
                                    nc.tensor.matmul(
                                        h3ps[ti][:, :wd], wt[:, k, 1, hs_],
                                        xgt[:, k % 4, off:off + wd],
                                        start=(k == 0), stop=(k == KD - 1),
                                    )
                            act = actp.tile([128, C], BF, tag="act", name="act")
                            for ti, (off, wd) in enumerate(T_TILES):
                                tmp = tmpp.tile([128, 512], F32, tag="tmp", name="tmp")
                                nc.scalar.activation(tmp[:, :wd], h1ps[ti][:, :wd], SIGMOID)
                                nc.vector.tensor_mul(
                                    tmp[:, :wd], tmp[:, :wd], h1ps[ti][:, :wd]
                                )
                                nc.vector.tensor_mul(
                                    act[:, off:off + wd], tmp[:, :wd], h3ps[ti][:, :wd]
                                )
                            act_sb.append(act)

                        # mm2: y_e^T[d,t] = sum_h w2-tile.T @ act
                        for d in range(KD):
                            w2s = w2p.tile([128, KH, 128], BF, tag="w2s", name="w2s")
                            nc.sync.dma_start(out=w2s, in_=w2q[e, d])
                            yps = [ps2.tile([128, 512], F32, tag="yps", name="yps")
                                   for _ in T_TILES]
                            for h in range(KH):
                                for ti, (off, wd) in enumerate(T_TILES):
                                    nc.tensor.matmul(
                                        yps[ti][:, :wd], w2s[:, h, :],
                                        act_sb[h][:, off:off + wd],
                                        start=(h == 0), stop=(h == KH - 1),
                                    )
                            for ti, (off, wd) in enumerate(T_TILES):
                                yo = youtp.tile([128, 512], F32, tag="yout", name="yout")
                                nc.vector.tensor_copy(yo[:, :wd], yps[ti][:, :wd])
                                nc.gpsimd.dma_start(
                                    out=yg[e, d * 128:(d + 1) * 128, off:off + wd],
                                    in_=yo[:, :wd],
                                )

                # ---------------- shared expert (TP shard) ----------------
                with (
                    tc.tile_pool(name="xshp", bufs=3) as xshp,
                    tc.tile_pool(name="ps1s", bufs=3, space="PSUM") as ps1s,
                    tc.tile_pool(name="ps2s", bufs=2, space="PSUM") as ps2s,
                ):
                    sw13_sb, sw2_sb = sw_sb_holder

                    def load_x(tb):
                        x_sb = xshp.tile([128, KD, 512], BF, tag="xsh", name="xsh")
                        nc.sync.dma_start(out=x_sb, in_=xq[tb])
                        return x_sb

                    def sh_mm1(x_sb):
                        sh_sb = []
                        for hs, (hoff, m) in enumerate(SH_HS):
                            h1ps = ps1s.tile([128, 512], F32, tag="h1s", name="h1s")
                            h3ps = ps1s.tile([128, 512], F32, tag="h3s", name="h3s")
                            hsl = slice(hoff, hoff + m)
                            for k in range(KD):
                                nc.tensor.matmul(
                                    h1ps[:m, :], sw13_sb[:, k, 0, hsl], x_sb[:, k, :],
                                    start=(k == 0), stop=(k == KD - 1),
                                )
                                nc.tensor.matmul(
                                    h3ps[:m, :], sw13_sb[:, k, 1, hsl], x_sb[:, k, :],
                                    start=(k == 0), stop=(k == KD - 1),
                                )
                            sh = actp.tile([128, 512], BF, tag="act", name="act")
                            tmp = tmpp.tile([128, 512], F32, tag="tmp", name="tmp")
                            nc.scalar.activation(tmp[:m, :], h1ps[:m, :], SIGMOID)
                            nc.vector.tensor_mul(tmp[:m, :], tmp[:m, :], h1ps[:m, :])
                            nc.vector.tensor_mul(sh[:m, :], tmp[:m, :], h3ps[:m, :])
                            sh_sb.append(sh)
                        return sh_sb

                    def sh_mm2(tb, sh_sb):
                        toff = tb * 512
                        for d in range(KD):
                            yps = ps2s.tile([128, 512], F32, tag="ypss", name="ypss")
                            dsl = slice(d * 128, (d + 1) * 128)
                            for hs, (hoff, m) in enumerate(SH_HS):
                                nc.tensor.matmul(
                                    yps, sw2_sb[:m, hs, dsl], sh_sb[hs][:m, :],
                                    start=(hs == 0), stop=(hs == 2),
                                )
                            yo = youtp.tile([128, 512], F32, tag="yout", name="yout")
                            if d % 2 == 0:
                                nc.scalar.copy(yo, yps)
                            else:
                                nc.vector.tensor_copy(yo, yps)
                            nc.gpsimd.dma_start(out=ysh[dsl, toff:toff + 512], in_=yo)

                    x_tiles = [load_x(0)]
                    for tb in range(NT):
                        if tb + 1 < NT:
                            x_tiles.append(load_x(tb + 1))
                        sh_mm2(tb, sh_mm1(x_tiles[tb]))

    nc.compile()
    _BUILD_CACHE[key] = nc
    return nc


def _routing(x2d, gate_w):
    """Host gate: top-2 of softmax(x @ gate_w.T), combine weights, aux."""
    scores = x2d @ gate_w.T                       # fp32
    m = scores.max(-1, keepdims=True)
    p = np.exp(scores - m, dtype=np.float32)
    p = p / p.sum(-1, keepdims=True)
    top2 = np.argsort(-p, axis=-1, kind="stable")[:, :TOPK]
    tv = np.take_along_axis(p, top2, axis=-1)
    tvn = tv / (tv.sum(-1, keepdims=True) + 1e-9)
    w = np.zeros((x2d.shape[0], E), np.float32)
    np.put_along_axis(w, top2, tvn.astype(np.float32), axis=-1)
    importance = p.mean(0)
    load = w.sum(0) / (w.sum() + np.float32(1e-9))
    aux = np.float32(E) * np.float32((importance * load).sum())
    return top2, tvn, w, aux


def _pmaj(a, kd):
    """[kd*128, F...] -> partition-major [128, kd, F...]"""
    return np.ascontiguousarray(a.reshape(kd, 128, *a.shape[1:]).swapaxes(0, 1))


def prepare(x, gate_w, w1, w2, w3, sw1, sw2, sw3, niter=1):
    """Host shard prep. Returns (in_maps, meta)."""
    x2d = np.asarray(x, np.float32).reshape(T, D)
    top2, tvn, w, aux = _routing(x2d, np.asarray(gate_w, np.float32))

    xTb = np.ascontiguousarray(x2d.T).astype(BF_NP)   # [D, T]

    idx_e, overflow = [], []
    for e in range(E):
        idx = np.nonzero((top2 == e).any(-1))[0]
        if len(idx) > C:
            overflow.append((e, idx[C:]))
            idx = idx[:C]
        idx_e.append(idx)

    # shared x, token-block-major: xq[tb, p, k, c] = x[tb*512+c, k*128+p]
    xq = np.ascontiguousarray(
        xTb.reshape(KD, 128, NT, 512).transpose(2, 1, 0, 3)
    )

    # sw13: [128, KD, 2, HSS]  (shared across cores except the shard slice)
    nit = np.array([[niter]], np.int32)
    in_maps = []
    for c in range(N_CORES):
        ge = [E_LOC * c + i for i in range(E_LOC)]
        gidx = np.zeros(E_LOC * C, np.int64)
        for i, g in enumerate(ge):
            gidx[i * C:i * C + len(idx_e[g])] = idx_e[g]
        xg = xTb[:, gidx]                               # [D, 2C]
        w13 = np.stack([
            np.stack([
                _pmaj(np.asarray(w1[g], np.float32).T.astype(BF_NP), KD),
                _pmaj(np.asarray(w3[g], np.float32).T.astype(BF_NP), KD),
            ], axis=2)
            for g in ge
        ])
        w2qa = np.stack([
            np.ascontiguousarray(
                np.asarray(w2[g], np.float32).T.astype(BF_NP)
                .reshape(KH, 128, KD, 128).transpose(2, 1, 0, 3)
            )
            for g in ge
        ])
        sw13 = np.stack([
            _pmaj(np.ascontiguousarray(
                np.asarray(m, np.float32)[c * HSS:(c + 1) * HSS, :].T
            ).astype(BF_NP), KD)
            for m in (sw1, sw3)
        ], axis=2)
        sw2t = np.vstack([
            np.ascontiguousarray(np.asarray(sw2, np.float32)[:, c * HSS:(c + 1) * HSS].T),
            np.zeros((3 * 128 - HSS, D), np.float32),
        ]).astype(BF_NP)
        sw2q = np.ascontiguousarray(sw2t.reshape(3, 128, D).swapaxes(0, 1))
        in_maps.append({
            "w13": w13,
            "w2q": w2qa,
            "xgq": _pmaj(xg, KD),
            "xq": xq,
            "sw13": sw13,
            "sw2q": sw2q,
            "niter": nit,
        })
    meta = dict(idx_e=idx_e, overflow=overflow, w=w, aux=aux, x2d=x2d)
    return in_maps, meta


def combine(results, meta, w1, w2, w3):
    idx_e, overflow, w = meta["idx_e"], meta["overflow"], meta["w"]
    y2d = np.zeros((T, D), np.float32)
    for c in range(N_CORES):
        y2d += results[c]["ysh"].T
        for i in range(E_LOC):
            g = E_LOC * c + i
            idx = idx_e[g]
            if len(idx) == 0:
                continue
            blk = results[c]["yg"][i][:, :len(idx)].T  # [cnt, D]
            y2d[idx] += blk * w[idx, g][:, None]
    # host fallback for capacity-overflow tokens (exact fp32)
    x2d = meta["x2d"]
    for g, idx in overflow:
        xs = x2d[idx]
        h1 = xs @ np.asarray(w1[g], np.float32).T
        h3 = xs @ np.asarray(w3[g], np.float32).T
        act = (h1 / (1.0 + np.exp(-h1))) * h3
        y2d[idx] += (act @ np.asarray(w2[g], np.float32).T) * w[idx, g][:, None]
    return y2d


def kernel(x, gate_w, w1, w2, w3, sw1, sw2, sw3):
    nc = build_nc()
    in_maps, meta = prepare(x, gate_w, w1, w2, w3, sw1, sw2, sw3, niter=1)
    res = run_bass_kernel_spmd(nc, in_maps, list(range(N_CORES)))
    y2d = combine(res.results, meta, w1, w2, w3)
    y = y2d.reshape(B, S, D)
    return y, meta["aux"]


# revision 19
# speedup vs baseline: 2.6283x; 1.0860x over previous
"""MoE (16 experts, top-2, shared SwiGLU expert) on 8 trn2 NeuronCores.

Strategy (expert-parallel, per sharding hint):
- Host computes the (cheap) gate: scores -> softmax -> top-2 -> combine
  weights. This determines the token->expert routing, i.e. the sharding.
- Each core c owns experts (2c, 2c+1). Tokens routed to an expert are
  gathered (host-side all-to-all) into a fixed-capacity [D, C] column
  matrix per expert, transposed so the contraction dim D sits on SBUF
  partitions.
- The shared expert is tensor-parallel over its hidden dim: core c owns
  rows [352c, 352(c+1)) of sw1/sw3 and the matching columns of sw2, and
  produces a [D, T] partial that the host sums over cores.
- Device math (per core): SwiGLU for 2 experts on their gathered tokens
  plus the shared-expert shard for all tokens. bf16 operands, fp32 PSUM
  accumulation. The per-token routing weight is applied host-side during
  the scatter-add (it commutes with the second matmul).
- Capacity overflow (count > C) is handled host-side for the few
  overflow tokens, so the kernel stays correct for any routing.

DMA-count discipline (HWDGE is a serialized ~625ns/DMA resource): every
input tensor is host-packed partition-major so it loads in one (or a
few) large multi-descriptor DMAs; output stores issue on the otherwise
idle GpSimd engine (SWDGE), keeping the HWDGE nearly free.

Outputs per core: yg [2, D, C] (routed, transposed), ysh [D, T] (shared
partial). Host: y = sum_c ysh_c^T + scatter-add of weighted yg columns.
"""

import contextlib

import numpy as np
import ml_dtypes

import concourse.bacc as bacc
import concourse.bass as bass
import concourse.mybir as mybir
import concourse.tile as tile
from concourse.bass_utils import run_bass_kernel_spmd

# problem shape (hardcoded per contract)
B, S, D = 2, 2048, 2048
T = B * S                  # 4096 tokens
E = 16                     # routed experts
TOPK = 2
H = 1408                   # expert hidden
HS = 2816                  # shared expert hidden
N_CORES = 8
E_LOC = E // N_CORES       # 2 experts per core
HSS = HS // N_CORES        # 352 shared-hidden rows per core

C = 512                    # per-expert token capacity (= one 512-wide tile;
                           # overflow tokens are handled exactly on the host)
KD = D // 128              # 16 contraction tiles over D
KH = H // 128              # 11 contraction tiles over H
NT = T // 512              # 8 token blocks for the shared expert
T_TILES = [(0, 512)]                       # routed token sub-tiles (C=512)
SH_HS = [(0, 128), (128, 128), (256, 96)]  # shared hidden sub-tiles (352)

BF = mybir.dt.bfloat16
F32 = mybir.dt.float32
BF_NP = ml_dtypes.bfloat16

_BUILD_CACHE = {}


def build_nc(timing_loop=True):
    """Build + compile the SPMD single-core program. Returns nc."""
    key = ("nc", timing_loop)
    if key in _BUILD_CACHE:
        return _BUILD_CACHE[key]

    nc = bacc.Bacc("TRN2", target_bir_lowering=False, debug=False)

    # host-packed, partition-major inputs (see prepare())
    w13 = nc.dram_tensor("w13", [E_LOC, 128, KD, 2, H], BF, kind="ExternalInput").ap()
    w2q = nc.dram_tensor("w2q", [E_LOC, KD, 128, KH, 128], BF, kind="ExternalInput").ap()
    xgq = nc.dram_tensor("xgq", [128, KD, E_LOC * C], BF, kind="ExternalInput").ap()
    xq = nc.dram_tensor("xq", [NT, 128, KD, 512], BF, kind="ExternalInput").ap()
    sw13 = nc.dram_tensor("sw13", [128, KD, 2, HSS], BF, kind="ExternalInput").ap()
    sw2q = nc.dram_tensor("sw2q", [128, 3, D], BF, kind="ExternalInput").ap()
    niter = nc.dram_tensor("niter", [1, 1], mybir.dt.int32, kind="ExternalInput").ap()

    yg = nc.dram_tensor("yg", [E_LOC, D, C], F32, kind="ExternalOutput").ap()
    ysh = nc.dram_tensor("ysh", [D, T], F32, kind="ExternalOutput").ap()

    SIGMOID = mybir.ActivationFunctionType.Sigmoid

    with tile.TileContext(nc) as tc:
        # runtime repeat count (timing support; harness runs with 1)
        if timing_loop:
            regs = []
            for ename, eng in nc.engines.items():
                r = eng.alloc_register(f"niter_{ename}")
                eng.reg_load(r, niter[0:1, 0:1])
                regs.append(r)
            rbound = bass.RegisterHandles(iter(regs))
            loop_cm = tc.For_i(0, rbound)
        else:
            loop_cm = contextlib.nullcontext()

        with loop_cm:
            with (
                tc.tile_pool(name="swp", bufs=1) as swp,
                tc.tile_pool(name="actp", bufs=13) as actp,
                tc.tile_pool(name="tmpp", bufs=3) as tmpp,
                tc.tile_pool(name="youtp", bufs=5) as youtp,
            ):
                # ---------------- routed experts ----------------
                with (
                    tc.tile_pool(name="w13p", bufs=1) as w13p,
                    tc.tile_pool(name="w2p", bufs=5) as w2p,
                    tc.tile_pool(name="xgp", bufs=1) as xgp,
                    tc.tile_pool(name="ps1", bufs=3, space="PSUM") as ps1,
                    tc.tile_pool(name="ps1b", bufs=3, space="PSUM") as ps1b,
                    tc.tile_pool(name="ps2", bufs=2, space="PSUM") as ps2,
                ):
                    HGRP = [(0, 1), (1, 3), (4, 4), (8, 3)]  # h-tile groups
                    xg_tiles = {}

                    def load_xg(e):
                        parts = []
                        for kh in range(4):
                            t = xgp.tile([128, KD // 4, C], BF,
                                         tag=f"xg{e}{kh}", name=f"xg{e}{kh}")
                            nc.sync.dma_start(
                                out=t,
                                in_=xgq[:, kh * 4:(kh + 1) * 4, e * C:(e + 1) * C],
                            )
                            parts.append(t)
                        xg_tiles[e] = parts

                    def load_w13_grp(e, gi):
                        g0, gn = HGRP[gi]
                        wtg = w13p.tile([128, KD, 2, gn * 128], BF,
                                        tag=f"w13{gi}", name=f"w13{gi}")
                        nc.sync.dma_start(
                            out=wtg,
                            in_=w13[e][:, :, :, g0 * 128:(g0 + gn) * 128],
                        )
                        return wtg

                    sw_sb_holder = []
                    # emit every weight/token load up front; pool slots and the
                    # scheduler pace the actual transfers
                    all_wts = []
                    for e in range(E_LOC):
                        wts = [load_w13_grp(e, 0)]
                        load_xg(e)
                        for gi in range(1, len(HGRP)):
                            wts.append(load_w13_grp(e, gi))
                        all_wts.append(wts)
                    t13 = swp.tile([128, KD, 2, HSS], BF, tag="sw13", name="sw13")
                    nc.sync.dma_start(out=t13, in_=sw13)
                    t2 = swp.tile([128, 3, D], BF, tag="sw2", name="sw2")
                    nc.sync.dma_start(out=t2, in_=sw2q)
                    sw_sb_holder[:] = [t13, t2]
                    for e in range(E_LOC):
                        wts = all_wts[e]

                        # mm1 + swiglu, [H, C] layout
                        act_sb = []
                        for h in range(KH):
                            h1ps = [ps1.tile([128, 512], F32, tag="h1", name="h1")
                                    for _ in T_TILES]
                            h3ps = [ps1b.tile([128, 512], F32, tag="h3", name="h3")
                                    for _ in T_TILES]
                            gi = next(i for i, (g0, gn) in enumerate(HGRP)
                                      if g0 <= h < g0 + gn)
                            wt = wts[gi]
                            hs_ = slice((h - HGRP[gi][0]) * 128,
                                        (h - HGRP[gi][0] + 1) * 128)
                            for k in range(KD):
                                xgt = xg_tiles[e][k // 4]
                                for ti, (off, wd) in enumerate(T_TILES):
                                    nc.tensor.matmul(
                                        h1ps[ti][:, :wd], wt[:, k, 0, hs_],
                                        xgt[:, k % 4, off:off + wd],
                                        start=(k == 0), stop=(k == KD - 1),
                                    )
                                for ti, (off, wd) in enumerate(T_TILES):
                                    nc.tensor.matmul(
                                        h3ps[ti][:, :wd], wt[:, k, 1, hs_],
                                        xgt[:, k % 4, off:off + wd],
                                        start=(k == 0), stop=(k == KD - 1),
                                    )
                            act = actp.tile([128, C], BF, tag="act", name="act")
                            for ti, (off, wd) in enumerate(T_TILES):
                                tmp = tmpp.tile([128, 512], F32, tag="tmp", name="tmp")
                                nc.scalar.activation(tmp[:, :wd], h1ps[ti][:, :wd], SIGMOID)
                                nc.vector.tensor_mul(
                                    tmp[:, :wd], tmp[:, :wd], h1ps[ti][:, :wd]
                                )
                                nc.vector.tensor_mul(
                                    act[:, off:off + wd], tmp[:, :wd], h3ps[ti][:, :wd]
                                )
                            act_sb.append(act)

                        # mm2: y_e^T[d,t] = sum_h w2-tile.T @ act
                        for d in range(KD):
                            w2s = w2p.tile([128, KH, 128], BF, tag="w2s", name="w2s")
                            nc.sync.dma_start(out=w2s, in_=w2q[e, d])
                            yps = [ps2.tile([128, 512], F32, tag="yps", name="yps")
                                   for _ in T_TILES]
                            for h in range(KH):
                                for ti, (off, wd) in enumerate(T_TILES):
                                    nc.tensor.matmul(
                                        yps[ti][:, :wd], w2s[:, h, :],
                                        act_sb[h][:, off:off + wd],
                                        start=(h == 0), stop=(h == KH - 1),
                                    )
                            for ti, (off, wd) in enumerate(T_TILES):
                                yo = youtp.tile([128, 512], F32, tag="yout", name="yout")
                                nc.vector.tensor_copy(yo[:, :wd], yps[ti][:, :wd])
                                nc.gpsimd.dma_start(
                                    out=yg[e, d * 128:(d + 1) * 128, off:off + wd],
                                    in_=yo[:, :wd],
                                )

                # ---------------- shared expert (TP shard) ----------------
                with (
                    tc.tile_pool(name="xshp", bufs=4) as xshp,
                    tc.tile_pool(name="ps1s", bufs=3, space="PSUM") as ps1s,
                    tc.tile_pool(name="ps2s", bufs=2, space="PSUM") as ps2s,
                ):
                    sw13_sb, sw2_sb = sw_sb_holder

                    def load_x(tb):
                        x_sb = xshp.tile([128, KD, 512], BF, tag="xsh", name="xsh")
                        nc.sync.dma_start(out=x_sb, in_=xq[tb])
                        return x_sb

                    def sh_mm1(x_sb):
                        sh_sb = []
                        for hs, (hoff, m) in enumerate(SH_HS):
                            h1ps = ps1s.tile([128, 512], F32, tag="h1s", name="h1s")
                            h3ps = ps1s.tile([128, 512], F32, tag="h3s", name="h3s")
                            hsl = slice(hoff, hoff + m)
                            for k in range(KD):
                                nc.tensor.matmul(
                                    h1ps[:m, :], sw13_sb[:, k, 0, hsl], x_sb[:, k, :],
                                    start=(k == 0), stop=(k == KD - 1),
                                )
                                nc.tensor.matmul(
                                    h3ps[:m, :], sw13_sb[:, k, 1, hsl], x_sb[:, k, :],
                                    start=(k == 0), stop=(k == KD - 1),
                                )
                            sh = actp.tile([128, 512], BF, tag="act", name="act")
                            tmp = tmpp.tile([128, 512], F32, tag="tmp", name="tmp")
                            nc.scalar.activation(tmp[:m, :], h1ps[:m, :], SIGMOID)
                            nc.vector.tensor_mul(tmp[:m, :], tmp[:m, :], h1ps[:m, :])
                            nc.vector.tensor_mul(sh[:m, :], tmp[:m, :], h3ps[:m, :])
                            sh_sb.append(sh)
                        return sh_sb

                    def sh_mm2(tb, sh_sb):
                        toff = tb * 512
                        for d in range(KD):
                            yps = ps2s.tile([128, 512], F32, tag="ypss", name="ypss")
                            dsl = slice(d * 128, (d + 1) * 128)
                            for hs, (hoff, m) in enumerate(SH_HS):
                                nc.tensor.matmul(
                                    yps, sw2_sb[:m, hs, dsl], sh_sb[hs][:m, :],
                                    start=(hs == 0), stop=(hs == 2),
                                )
                            yo = youtp.tile([128, 512], F32, tag="yout", name="yout")
                            if d % 2 == 0:
                                nc.scalar.copy(yo, yps)
                            else:
                                nc.vector.tensor_copy(yo, yps)
                            nc.gpsimd.dma_start(out=ysh[dsl, toff:toff + 512], in_=yo)

                    x_tiles = [load_x(0)]
                    for tb in range(NT):
                        if tb + 1 < NT:
                            x_tiles.append(load_x(tb + 1))
                        sh_mm2(tb, sh_mm1(x_tiles[tb]))

    nc.compile()
    _BUILD_CACHE[key] = nc
    return nc


def _routing(x2d, gate_w):
    """Host gate: top-2 of softmax(x @ gate_w.T), combine weights, aux."""
    scores = x2d @ gate_w.T                       # fp32
    m = scores.max(-1, keepdims=True)
    p = np.exp(scores - m, dtype=np.float32)
    p = p / p.sum(-1, keepdims=True)
    top2 = np.argsort(-p, axis=-1, kind="stable")[:, :TOPK]
    tv = np.take_along_axis(p, top2, axis=-1)
    tvn = tv / (tv.sum(-1, keepdims=True) + 1e-9)
    w = np.zeros((x2d.shape[0], E), np.float32)
    np.put_along_axis(w, top2, tvn.astype(np.float32), axis=-1)
    importance = p.mean(0)
    load = w.sum(0) / (w.sum() + np.float32(1e-9))
    aux = np.float32(E) * np.float32((importance * load).sum())
    return top2, tvn, w, aux


def _pmaj(a, kd):
    """[kd*128, F...] -> partition-major [128, kd, F...]"""
    return np.ascontiguousarray(a.reshape(kd, 128, *a.shape[1:]).swapaxes(0, 1))


def prepare(x, gate_w, w1, w2, w3, sw1, sw2, sw3, niter=1):
    """Host shard prep. Returns (in_maps, meta)."""
    x2d = np.asarray(x, np.float32).reshape(T, D)
    top2, tvn, w, aux = _routing(x2d, np.asarray(gate_w, np.float32))

    xTb = np.ascontiguousarray(x2d.T).astype(BF_NP)   # [D, T]

    idx_e, overflow = [], []
    for e in range(E):
        idx = np.nonzero((top2 == e).any(-1))[0]
        if len(idx) > C:
            overflow.append((e, idx[C:]))
            idx = idx[:C]
        idx_e.append(idx)

    # shared x, token-block-major: xq[tb, p, k, c] = x[tb*512+c, k*128+p]
    xq = np.ascontiguousarray(
        xTb.reshape(KD, 128, NT, 512).transpose(2, 1, 0, 3)
    )

    # sw13: [128, KD, 2, HSS]  (shared across cores except the shard slice)
    nit = np.array([[niter]], np.int32)
    in_maps = []
    for c in range(N_CORES):
        ge = [E_LOC * c + i for i in range(E_LOC)]
        gidx = np.zeros(E_LOC * C, np.int64)
        for i, g in enumerate(ge):
            gidx[i * C:i * C + len(idx_e[g])] = idx_e[g]
        xg = xTb[:, gidx]                               # [D, 2C]
        w13 = np.stack([
            np.stack([
                _pmaj(np.asarray(w1[g], np.float32).T.astype(BF_NP), KD),
                _pmaj(np.asarray(w3[g], np.float32).T.astype(BF_NP), KD),
            ], axis=2)
            for g in ge
        ])
        w2qa = np.stack([
            np.ascontiguousarray(
                np.asarray(w2[g], np.float32).T.astype(BF_NP)
                .reshape(KH, 128, KD, 128).transpose(2, 1, 0, 3)
            )
            for g in ge
        ])
        sw13 = np.stack([
            _pmaj(np.ascontiguousarray(
                np.asarray(m, np.float32)[c * HSS:(c + 1) * HSS, :].T
            ).astype(BF_NP), KD)
            for m in (sw1, sw3)
        ], axis=2)
        sw2t = np.vstack([
            np.ascontiguousarray(np.asarray(sw2, np.float32)[:, c * HSS:(c + 1) * HSS].T),
            np.zeros((3 * 128 - HSS, D), np.float32),
        ]).astype(BF_NP)
        sw2q = np.ascontiguousarray(sw2t.reshape(3, 128, D).swapaxes(0, 1))
        in_maps.append({
            "w13": w13,
            "w2q": w2qa,
            "xgq": _pmaj(xg, KD),
            "xq": xq,
            "sw13": sw13,
            "sw2q": sw2q,
            "niter": nit,
        })
    meta = dict(idx_e=idx_e, overflow=overflow, w=w, aux=aux, x2d=x2d)
    return in_maps, meta


def combine(results, meta, w1, w2, w3):
    idx_e, overflow, w = meta["idx_e"], meta["overflow"], meta["w"]
    y2d = np.zeros((T, D), np.float32)
    for c in range(N_CORES):
        y2d += results[c]["ysh"].T
        for i in range(E_LOC):
            g = E_LOC * c + i
            idx = idx_e[g]
            if len(idx) == 0:
                continue
            blk = results[c]["yg"][i][:, :len(idx)].T  # [cnt, D]
            y2d[idx] += blk * w[idx, g][:, None]
    # host fallback for capacity-overflow tokens (exact fp32)
    x2d = meta["x2d"]
    for g, idx in overflow:
        xs = x2d[idx]
        h1 = xs @ np.asarray(w1[g], np.float32).T
        h3 = xs @ np.asarray(w3[g], np.float32).T
        act = (h1 / (1.0 + np.exp(-h1))) * h3
        y2d[idx] += (act @ np.asarray(w2[g], np.float32).T) * w[idx, g][:, None]
    return y2d


def kernel(x, gate_w, w1, w2, w3, sw1, sw2, sw3):
    nc = build_nc()
    in_maps, meta = prepare(x, gate_w, w1, w2, w3, sw1, sw2, sw3, niter=1)
    res = run_bass_kernel_spmd(nc, in_maps, list(range(N_CORES)))
    y2d = combine(res.results, meta, w1, w2, w3)
    y = y2d.reshape(B, S, D)
    return y, meta["aux"]
